# revision 3
# baseline (speedup 1.0000x reference)
"""Multi-head attention with "restricted softmax" on 8 TRN2 NeuronCores.

Reference computation (per head):
    score = Q @ K.T / sqrt(D)                       # [S, S]
    attn  = exp(score) / (1 + sum_k exp(score))     # restricted softmax
            (mathematically identical to the max-clamped reference form)
    out   = attn @ V                                # [S, D]

Full problem: B=2, H=16, S=2048, D=64  ->  32 heads, 4 heads per core.

The ScalarEngine's exp is the hard floor (1 elem/cycle/lane @ 1.2 GHz,
(N+352)/1.2 ns per instruction), so everything is built to keep it
saturated with the widest ACTIVATEs PSUM allows:
  - Scores computed TRANSPOSED (S^T[k, q]) in [128, 512] units; THREE
    units share one ACTIVATE (N=1536 -> 1.025 ns/elem vs 1.12 at N=1024):
    6 PSUM banks double-buffered + 2 banks for the PV accumulator.
  - Scores matmuls contract K=64 directly (no zero-padding: back-to-back
    K=64 matmuls stream at the same ~216 ns/512-col as K=128).
  - PV uses lhsT=[V | 1] so PSUM row 64 accumulates sum_k exp (the
    softmax denominator) for free.
  - Steady-state epilogue (normalize + [d,q]->[q,d]) runs entirely off
    the PE: fp16 bounce to DRAM + X-bar transpose back (chunk-major 3D
    dest). Every stage is DEFERRED a few units so its dependency is
    already satisfied at dispatch: the DVE / Sync / GpSimd queues are
    in-order and a waiting op would head-of-line-block the whole stream.
  - DMA issue is split across queues: X-bar transposes on Sync (HWDGE),
    steady-state staging + epilogue DMAs on the otherwise-idle GpSimd
    (SWDGE), ramp DMAs on Sync while GpSimd/PE are empty.
  - Head 0 ramp: chunked loads + PE transposes (PE is idle then); the
    exp table is pre-warmed so ACT_TABLE_LOAD overlaps the DMA ramp.
    Last pass epilogue also uses PE transposes to cut the tail.
"""

import heapq
import os

import numpy as np

import concourse.bass as bass  # noqa: F401  (bass must import before tile)
import concourse.mybir as mybir
import concourse.tile as tile
from concourse import bacc
from concourse.bass_utils import run_bass_kernel_spmd
from concourse.masks import make_identity

B, H, S, D = 2, 16, 2048, 64
N_CORES = 8
HPC = (B * H) // N_CORES  # heads per core = 4

F32 = mybir.dt.float32
F16 = mybir.dt.float16
EXP = mybir.ActivationFunctionType.Exp

SCALE = 1.0 / 8.0   # 1/sqrt(D)
NK = S // 128       # 16 k-tiles of 128
QB = 512            # q-block width per pass
NQB = S // QB       # 4 q-blocks per head
NPASS = HPC * NQB   # 16 passes
UNITS = NPASS * NK  # 256 scores units of [128k, 512q]
GRP = 3             # units per ACTIVATE group


class _HeadInputs:
    """Per-head staged inputs: fp16 Q^T/K^T [128, S] (only rows 0..63 are
    real data; scores contract K=64) and [V | 1] fp16.

    Heads 1..3: DMA X-bar transpose of an fp16 bounce buffer in DRAM
    (zero PE cost), with each stage emitted as a deferred action so no
    queue ever dispatches a waiting op. Head 0: PE transposes JIT'd into
    the idle ramp."""

    def __init__(self, ctx, h):
        self.ctx = ctx
        self.h = h
        self.ready = {"q": set(), "k": set()}  # head-0 JIT transpose state

    def _alloc(self):
        pools, h = self.ctx, self.h
        hp = pools["head_pool"]
        self.q_nat = hp.tile([128, NK, D], F32, tag="q_nat", name=f"q_nat{h}")
        self.k_nat = hp.tile([128, NK, D], F32, tag="k_nat", name=f"k_nat{h}")
        self.v_nat = hp.tile([128, NK, D], F32, tag="v_nat", name=f"v_nat{h}")
        # fp16 staging; cols 64..127 are never written nor read
        self.q16 = hp.tile([128, NK, 128], F16, tag="q16", name=f"q16_{h}")
        self.k16 = hp.tile([128, NK, 128], F16, tag="k16", name=f"k16_{h}")
        self.v1 = hp.tile([128, NK, D + 1], F16, tag="v1", name=f"v1_{h}")
        self.qT = pools["qkt_pool"].tile([128, S], F16, tag="qT", name=f"qT{h}")
        self.kT = pools["qkt_pool"].tile([128, S], F16, tag="kT", name=f"kT{h}")

    # ---- heads 1..3: four deferred stages ----
    def stage_nat(self):
        nc, pools = self.ctx["nc"], self.ctx
        self._alloc()
        for nat, src in (
            (self.k_nat, pools["k_dram"]), (self.q_nat, pools["q_dram"]),
            (self.v_nat, pools["v_dram"]),
        ):
            nc.gpsimd.dma_start(
                nat[:], src[self.h].rearrange("(n p) d -> p n d", p=128)
            )

    def stage_cast(self):
        nc, pools = self.ctx["nc"], self.ctx
        nc.vector.tensor_copy(self.k16[:, :, :D], self.k_nat[:])
        nc.vector.tensor_copy(self.q16[:, :, :D], self.q_nat[:])
        nc.vector.tensor_copy(
            self.v1[:, :, D:].rearrange("p n one -> p (n one)"), pools["ones"][:]
        )
        nc.vector.tensor_copy(self.v1[:, :, :D], self.v_nat[:])

    def stage_bounce(self):
        nc, pools, h = self.ctx["nc"], self.ctx, self.h
        dp = pools["dram_pool"]
        self.qdr = dp.tile([S, 128], F16, tag="qdr", name=f"qdr{h}")
        self.kdr = dp.tile([S, 128], F16, tag="kdr", name=f"kdr{h}")
        for dr, st16 in ((self.kdr, self.k16), (self.qdr, self.q16)):
            nc.gpsimd.dma_start(
                dr[:].rearrange("(n p) c -> p n c", p=128), st16[:]
            )

    def stage_xbar(self):
        nc = self.ctx["nc"]
        nc.sync.dma_start_transpose(self.kT[:], self.kdr[:])
        nc.sync.dma_start_transpose(self.qT[:], self.qdr[:])

    # ---- head 0: chunked ramp + JIT PE transposes ----
    def ramp_dma(self):
        nc, pools = self.ctx["nc"], self.ctx
        self._alloc()
        chunks = ((0, 4), (4, 12))  # n-block ranges: small first chunk
        for n0, nn in chunks:
            ns = slice(n0, n0 + nn)
            for nat, st16, src in (
                (self.k_nat, self.k16, pools["k_dram"]),
                (self.q_nat, self.q16, pools["q_dram"]),
            ):
                nc.sync.dma_start(
                    nat[:, ns, :],
                    src[0].rearrange("(n p) d -> p n d", p=128)[:, ns, :],
                )
                nc.vector.tensor_copy(st16[:, ns, :D], nat[:, ns, :])
            if n0 == 0:
                nc.sync.dma_start(
                    self.v_nat[:],
                    pools["v_dram"][0].rearrange("(n p) d -> p n d", p=128),
                )
        nc.vector.tensor_copy(
            self.v1[:, :, D:].rearrange("p n one -> p (n one)"), pools["ones"][:]
        )
        nc.vector.tensor_copy(self.v1[:, :, :D], self.v_nat[:])

    def ensure_h0(self, kind, n):
        """JIT a [64, 128] PE transpose of staging block n into qT/kT."""
        if n in self.ready[kind]:
            return
        self.ready[kind].add(n)
        nc, pools = self.ctx["nc"], self.ctx
        st16, tT = (self.q16, self.qT) if kind == "q" else (self.k16, self.kT)
        tp = pools["ps_o_pool"].tile([D, 128], F16, tag="oT", name="tp")
        nc.tensor.transpose(tp[:], st16[:, n, :D], pools["ident16"][:])
        nc.vector.tensor_copy(tT[:D, n * 128:(n + 1) * 128], tp[:])


def _attention(tc):
    nc = tc.nc
    q_dram = nc.dram_tensor("query", [HPC, S, D], F32, kind="ExternalInput").ap()
    k_dram = nc.dram_tensor("key", [HPC, S, D], F32, kind="ExternalInput").ap()
    v_dram = nc.dram_tensor("value", [HPC, S, D], F32, kind="ExternalInput").ap()
    o_dram = nc.dram_tensor("out", [HPC, S, D], F32, kind="ExternalOutput").ap()

    with (
        tc.tile_pool(name="const", bufs=1) as const_pool,
        tc.tile_pool(name="head_io", bufs=2) as head_pool,
        tc.tile_pool(name="qkt", bufs=2) as qkt_pool,
        tc.tile_pool(name="et", bufs=2) as et_pool,
        tc.tile_pool(name="epi", bufs=2) as epi_pool,
        tc.tile_pool(name="dram", bufs=2, space="DRAM") as dram_pool,
        tc.tile_pool(name="dram_epi", bufs=2, space="DRAM") as dram_epi_pool,
        tc.tile_pool(name="ps_g", bufs=2, space="PSUM") as ps_g_pool,
        tc.tile_pool(name="ps_o", bufs=2, space="PSUM") as ps_o_pool,
    ):
        ones = const_pool.tile([128, NK], F16)
        nc.vector.memset(ones[:], 1.0)
        # pre-warm the exp table so ACT_TABLE_LOAD overlaps the DMA ramp
        warm = const_pool.tile([128, 1], F16)
        nc.vector.memset(warm[:], 0.0)
        nc.scalar.activation(warm[:], warm[:], EXP)
        ident16 = const_pool.tile([128, 128], F16)
        make_identity(nc, ident16[:])

        ctx = {
            "nc": nc, "q_dram": q_dram, "k_dram": k_dram, "v_dram": v_dram,
            "head_pool": head_pool, "qkt_pool": qkt_pool,
            "dram_pool": dram_pool, "ps_o_pool": ps_o_pool,
            "ones": ones, "ident16": ident16,
        }

        heads = [_HeadInputs(ctx, h) for h in range(HPC)]
        heads[0].ramp_dma()

        # deferred-action scheduler, keyed by unit index
        actions = []
        aseq = [0]

        def defer(due_u, fn):
            heapq.heappush(actions, (due_u, aseq[0], fn))
            aseq[0] += 1

        def run_due(u):
            while actions and actions[0][0] <= u:
                heapq.heappop(actions)[2]()

        def emit_scores(units):
            slot = ps_g_pool.tile([128, GRP, QB], F32, tag="s", name="s")
            for j, u in enumerate(units):
                p, k = divmod(u, NK)
                h, qb = divmod(p, NQB)
                hd = heads[h]
                if h == 0:
                    hd.ensure_h0("k", k)
                    for nb in range(qb * 4, qb * 4 + 4):
                        hd.ensure_h0("q", nb)
                nc.tensor.matmul(
                    slot[:, j, :],
                    hd.kT[:64, k * 128:(k + 1) * 128],
                    hd.qT[:64, qb * QB:(qb + 1) * QB],
                    start=True, stop=True,
                )
            return slot

        def emit_epilogue(h, qb, oT, u_end):
            """Normalize + un-transpose oT [65,512] -> out [512,64], fully
            off-PE, each stage deferred past its dependency's completion."""
            oT16 = epi_pool.tile([80, QB], F16, tag="oT16", name="oT16")
            odr = dram_epi_pool.tile([80, QB], F16, tag="odr", name="odr")
            tr = epi_pool.tile([128, 4, 80], F16, tag="tr", name="tr")

            def stage_a():
                nc.vector.tensor_copy(oT16[:65, :], oT[:])

            def stage_b():
                nc.gpsimd.dma_start(odr[:65, :], oT16[:65, :])

            def stage_c():
                nc.sync.dma_start_transpose(tr[:], odr[:])

            def stage_d():
                den = epi_pool.tile([128, 4], F32, tag="den", name="den")
                nc.vector.tensor_scalar_add(den[:], tr[:, :, D], 1.0)
                rec = epi_pool.tile([128, 4], F32, tag="rec", name="rec")
                nc.vector.reciprocal(rec[:], den[:])
                o_sb = epi_pool.tile([128, 4, D], F32, tag="o_sb", name="o_sb")
                for j in range(4):
                    nc.vector.tensor_scalar_mul(
                        o_sb[:, j, :], tr[:, j, :D], rec[:, j:j + 1]
                    )
                nc.gpsimd.dma_start(
                    o_dram[h].rearrange("(n p) d -> p n d", p=128)[:, qb * 4:qb * 4 + 4, :],
                    o_sb[:],
                )

            defer(u_end + 2, stage_a)
            defer(u_end + 5, stage_b)
            defer(u_end + 8, stage_c)
            defer(u_end + 11, stage_d)

        def emit_tail_epilogue(h, qb, oT):
            """Last pass: PE transposes (PE is idle) — shortest tail."""
            oT16 = epi_pool.tile([80, QB], F16, tag="oT16", name="oT16")
            nc.vector.tensor_copy(oT16[:65, :], oT[:])
            tp = ps_o_pool.tile([128, 4, 68], F16, tag="oT", name="tp_tail")
            for j in range(4):
                nc.tensor.transpose(
                    tp[:, j, :65], oT16[:65, j * 128:(j + 1) * 128],
                    ident16[:65, :65],
                )
            den = epi_pool.tile([128, 4], F32, tag="den", name="den")
            nc.vector.tensor_scalar_add(den[:], tp[:, :, D], 1.0)
            rec = epi_pool.tile([128, 4], F32, tag="rec", name="rec")
            nc.vector.reciprocal(rec[:], den[:])
            o_sb = epi_pool.tile([128, 4, D], F32, tag="o_sb", name="o_sb")
            for j in range(4):
                nc.vector.tensor_scalar_mul(
                    o_sb[:, j, :], tp[:, j, :D], rec[:, j:j + 1]
                )
            nc.sync.dma_start(
                o_dram[h].rearrange("(n p) d -> p n d", p=128)[:, qb * 4:qb * 4 + 4, :],
                o_sb[:],
            )

        groups = [
            list(range(gs, min(gs + GRP, UNITS))) for gs in range(0, UNITS, GRP)
        ]
        slot_cur = emit_scores(groups[0])
        oT = None
        for g, units in enumerate(groups):
            w = len(units)
            et = et_pool.tile([128, GRP, QB], F16, tag="et", name="et")
            nc.scalar.activation(
                et[:, :w, :], slot_cur[:, :w, :], EXP, scale=SCALE
            )
            if g + 1 < len(groups):
                slot_cur = emit_scores(groups[g + 1])
            for j, u in enumerate(units):
                run_due(u)
                p, k = divmod(u, NK)
                h, qb = divmod(p, NQB)
                if k == 0:
                    oT = ps_o_pool.tile([65, QB], F32, tag="oT", name="oT")
                    # stage the next head across the following ~9 units
                    if qb == 1 and h + 1 < HPC:
                        hd_n = heads[h + 1]
                        defer(u + 0, hd_n.stage_nat)
                        defer(u + 3, hd_n.stage_cast)
                        defer(u + 6, hd_n.stage_bounce)
                        defer(u + 9, hd_n.stage_xbar)
                nc.tensor.matmul(
                    oT[:], heads[h].v1[:, k, :], et[:, j, :],
                    start=(k == 0), stop=(k == NK - 1),
                )
                if k == NK - 1:
                    if p == NPASS - 1:
                        emit_tail_epilogue(h, qb, oT)
                    else:
                        emit_epilogue(h, qb, oT, u)
        while actions:
            heapq.heappop(actions)[2]()


_NC_CACHE = None
_TRACE_READY = False


def _enable_tracing():
    """Register the NTFF profile hook that this image's antenv lacks, and
    keep profiling artifacts local instead of uploading to a bucket."""
    global _TRACE_READY
    if _TRACE_READY:
        return
    import sys
    import types

    import antenv
    import concourse.bass_utils as bu
    from trn_agent_boot.trn_boot import _ntff_profile_via_ctypes

    if "antenv.axon_hooks" not in sys.modules:
        mod = types.ModuleType("antenv.axon_hooks")
        mod._hook = None

        def set_axon_ntff_profile_hook(h):
            mod._hook = h

        def get_axon_ntff_profile_hook():
            return mod._hook

        mod.set_axon_ntff_profile_hook = set_axon_ntff_profile_hook
        mod.get_axon_ntff_profile_hook = get_axon_ntff_profile_hook
        sys.modules["antenv.axon_hooks"] = mod
        antenv.axon_hooks = mod

    hooks = sys.modules["antenv.axon_hooks"]
    if hooks.get_axon_ntff_profile_hook() is None:
        hooks.set_axon_ntff_profile_hook(
            _ntff_profile_via_ctypes("/opt/axon/libaxon_pjrt.so")
        )
    bu.upload_artifacts = lambda tmpdir: tmpdir
    _TRACE_READY = True


def _build():
    global _NC_CACHE
    if _NC_CACHE is None:
        nc = bacc.Bacc("TRN2", target_bir_lowering=False, debug=False)
        with tile.TileContext(nc) as tc:
            _attention(tc)
        nc.compile()
        _NC_CACHE = nc
    return _NC_CACHE


def _run(query, key, value, trace=False, tmpdir=None):
    if trace:
        _enable_tracing()
    q = np.ascontiguousarray(np.asarray(query, dtype=np.float32).reshape(B * H, S, D))
    k = np.ascontiguousarray(np.asarray(key, dtype=np.float32).reshape(B * H, S, D))
    v = np.ascontiguousarray(np.asarray(value, dtype=np.float32).reshape(B * H, S, D))
    in_maps = [
        {
            "query": q[c * HPC:(c + 1) * HPC],
            "key": k[c * HPC:(c + 1) * HPC],
            "value": v[c * HPC:(c + 1) * HPC],
        }
        for c in range(N_CORES)
    ]
    nc = _build()
    res = run_bass_kernel_spmd(
        nc, in_maps, core_ids=list(range(N_CORES)), trace=trace, tmpdir=tmpdir
    )
    out = np.stack([res.results[c]["out"] for c in range(N_CORES)])  # [8, HPC, S, D]
    return out.reshape(B, H, S, D), res


def kernel(query, key, value):
    out, _ = _run(query, key, value, trace=bool(int(os.environ.get("BASS_TRACE", "0"))))
    return out
